# revision 5
# baseline (speedup 1.0000x reference)
"""ArcFace multi-head-sharded loss on 8 TRN2 NeuronCores.

Strategy: shard the (64, 2048, 256) weight table over the group axis —
each core owns 8 groups. Samples are routed host-side to the core owning
their group (host routing replaces the all-to-all). Weight rows are
l2-normalized host-side and quantized to fp8e4 (x16 pre-scale to stay in
the normal range); embeddings stay bf16 (PE runs mixed bf16 x fp8 at
full column rate). The device does:

  - stream its 8 weight groups (4MB fp8) from HBM on the sync HWDGE ring
    as 16 uniform 256KB pieces in chunk-major order (band-pair x
    512-class chunk, 2KB contiguous per partition), so each (tile,
    chunk) unlocks progressively and the compute pipeline chases the
    stream with a ~3us tail,
  - mains: psum(b, c) = <x_b, wq_c> on PE. Four 32-sample bands sit in
    the four column quadrants of the array (tile_position); the j-loop
    is innermost within each (chunk, k) group so the four quadrant
    matmuls overlap (~4ns issue stagger). A short warm-up burst flips
    the HAM clock gate to 2.4 GHz before the mains arrive,
  - exp per 512-class chunk with the class-axis sum fused via accum_out
    (scale folds the 1/16 weight quantization scale and the ArcFace
    scale 64),
  - lb = ln(a*sum + b) where (a, b) fold the entire ArcFace margin +
    target-logit swap, precomputed host-side per sample,
  - one [128,1] f32 column out per sample tile (all but the last tile's
    output DMA hide under the weight stream).

Host: sums the lb of valid rows across cores, /B + SHIFT. ~4MB HBM
traffic per core => memory-bound.

Samples are packed into "bands" of NG=32 partition rows, one band per
weight group (plus overflow bands), 4 bands per 128-row sample tile.
"""

import sys
import numpy as np
import ml_dtypes

BF16 = ml_dtypes.bfloat16
FP8 = ml_dtypes.float8_e4m3

_TRN_REPO = "/opt/trn_rl_repo"
if _TRN_REPO not in sys.path:
    sys.path.insert(0, _TRN_REPO)

# problem config (hardcoded per spec)
B, E, G, C = 512, 256, 64, 2048
NCORES = 8
GPC = G // NCORES        # weight groups per core
NG = 32                  # sample slots per band
BPT = 128 // NG          # bands per 128-partition sample tile
KE = E // 128            # contraction chunks
NCC = C // 512           # 512-col chunks per group
SCALE = 64.0
MARGIN = 0.5
COS_M = float(np.cos(MARGIN))
SIN_M = float(np.sin(MARGIN))
THETA = float(np.cos(np.pi - MARGIN))
SINMM = float(np.sin(np.pi - MARGIN) * MARGIN)
EPS = 1e-12
WS = 16.0                # fp8 weight pre-scale; PSUM = WS * cos (x is bf16)
ESCALE = SCALE / WS
LB_SHIFT = float(40.0 * np.log(2.0))  # ln-range shift, re-added host-side
NWARM = 6                # PE warm-up matmuls (HAM throttle release)

_graph_cache = {}


def _build(nb):
    """Build the per-core Bass graph for nb weight bands (nb % BPT == 0)."""
    from contextlib import ExitStack
    import concourse.bacc as bacc
    import concourse.tile as tile
    from concourse import mybir

    f32 = mybir.dt.float32
    bf16 = mybir.dt.bfloat16
    fp8 = mybir.dt.float8e4
    AF = mybir.ActivationFunctionType

    T = nb // BPT
    NP2 = nb // 2            # band pairs
    NPC = NP2 * NCC          # 256KB stream pieces, chunk-major
    nc = bacc.Bacc(None)

    # weight stream: piece i = (chunk cc = i // NP2, band pair pr = i % NP2),
    # per partition [2 bands, KE*512] contiguous (2KB)
    wt_ext = nc.declare_dram_parameter("wt", [128, NPC, 2, KE * 512], fp8,
                                       isOutput=False)
    xtq_ext = nc.declare_dram_parameter("xtq", [128, T * KE * 128], bf16,
                                        isOutput=False)
    ab_ext = nc.declare_dram_parameter("ab", [128, 2 * T], f32, isOutput=False)
    out_ext = nc.declare_dram_parameter("out", [128, T], f32, isOutput=True)

    with tile.TileContext(nc) as tc, ExitStack() as ctx:
        wpool = ctx.enter_context(tc.tile_pool(name="w", bufs=NPC))
        cpool = ctx.enter_context(tc.tile_pool(name="const", bufs=1))
        vpool = ctx.enter_context(tc.tile_pool(name="vec", bufs=2))
        pmain = ctx.enter_context(tc.tile_pool(name="pmain", bufs=8, space="PSUM"))

        p_tiles = [wpool.tile([128, 2, KE, 512], fp8, tag="wt", name=f"wt{i}")
                   for i in range(NPC)]
        xtq_sb = cpool.tile([128, T * KE * 128], bf16, tag="xtq")
        ab_sb = cpool.tile([128, 2 * T], f32, tag="ab")
        lb_sb = cpool.tile([128, T], f32, tag="lb")

        # tiny inputs ride the scalar HWDGE ring so the weight stream on
        # the sync ring is never interrupted
        nc.scalar.dma_start(out=xtq_sb[:], in_=xtq_ext[:])
        nc.scalar.dma_start(out=ab_sb[:], in_=ab_ext[:])

        # the weight stream, chunk-major so (tile, chunk) compute unlocks
        # progressively while later chunks are still in flight
        for i in range(NPC):
            nc.sync.dma_start(out=p_tiles[i][:], in_=wt_ext[:, i])

        # exp scale as a per-partition AP (matches the fast ACT path; an
        # immediate scale measured ~2x slower per column)
        escale = cpool.tile([128, 1], f32, tag="escale")
        nc.vector.memset(escale[:], ESCALE)

        cps = {(t, cc): pmain.tile([128, 512], f32, tag="cos",
                                   name=f"cos{t}_{cc}")
               for t in range(T) for cc in range(NCC)}
        ses_t = {t: cpool.tile([128, NCC], f32, tag=f"ses{t}", name=f"ses{t}")
                 for t in range(T)}

        # PE warm-up: stream bf16 columns through the array so the HAM
        # activity window flips the clock gate to 2.4 GHz before the real
        # mains arrive (~3.4 us of sustained activity needed)
        if NWARM:
            warm_s = cpool.tile([128, 1], bf16, tag="warm_s")
            warm_m = cpool.tile([128, 512], bf16, tag="warm_m")
            nc.vector.memset(warm_s[:], 0.0)
            nc.vector.memset(warm_m[:], 0.0)
            for _ in range(NWARM):
                nc.tensor.matmul(cps[(0, 0)][0:1, :], warm_s[:], warm_m[:],
                                 start=True, stop=True)

        def mm(t, cc, k, j):
            pr = 2 * t + j // 2          # band pair of band 4t+j
            piece = p_tiles[cc * NP2 + pr]
            nc.tensor.matmul(
                cps[(t, cc)][NG * j:NG * (j + 1), :],
                xtq_sb[:, 256 * t + 128 * k + NG * j:
                       256 * t + 128 * k + NG * (j + 1)],
                piece[:, j % 2, k, :],
                start=(k == 0), stop=(k == KE - 1),
                tile_position=(0, NG * j),
            )

        def emit_exp(t, cc):
            """PSUM chunk -> bf16 SBUF via DVE (frees the bank, feeds ACT
            its fast input path), then exp with the class-axis sum fused
            via accum_out."""
            cb = cpool.tile([128, 512], bf16, tag=f"cosbf{t}_{cc}",
                            name=f"cosbf{t}_{cc}")
            nc.vector.tensor_copy(cb[:], cps[(t, cc)][:])
            escr = vpool.tile([128, 512], bf16, tag="escr")
            nc.scalar.activation(escr[:], cb[:], AF.Exp, scale=escale[:],
                                 accum_out=ses_t[t][:, cc:cc + 1])

        def emit_tail(t):
            """lb = ln(a * sum_cc ses + b); (a, b) fold margin + swap."""
            sfull = cpool.tile([128, 1], f32, tag=f"sfull{t}")
            nc.vector.reduce_sum(sfull[:], ses_t[t][:], axis=mybir.AxisListType.X)
            nc.scalar.activation(lb_sb[:, t:t + 1], sfull[:], AF.Ln,
                                 scale=ab_sb[:, t:t + 1],
                                 bias=ab_sb[:, T + t:T + t + 1])
            nc.scalar.dma_start(out=out_ext[:, t:t + 1], in_=lb_sb[:, t:t + 1])

        # chunk-major emission matching the stream order: (t, cc) chunks
        # complete every other piece; j innermost within each (cc, k) so
        # the four quadrant matmuls overlap (~4ns issue stagger)
        for cc in range(NCC):
            for t in range(T):
                for k in range(KE):
                    for j in range(BPT):
                        mm(t, cc, k, j)
                emit_exp(t, cc)
                if cc == NCC - 1:
                    emit_tail(t)

    nc.compile()
    return nc


def _pack(logits, labels, weight):
    """Route samples to the core owning their group; build per-core inputs."""
    logits = np.asarray(logits, dtype=np.float32)
    labels = np.asarray(labels).astype(np.int64)
    weight = np.asarray(weight, dtype=np.float32)

    group = (labels // C).astype(np.int64)
    local = (labels % C).astype(np.int64)
    core = group // GPC
    gl = group % GPC

    # host-side l2 normalization; weights quantized to fp8 (x16 keeps
    # values in fp8e4's normal range; cos is invariant to row scaling),
    # x stays bf16 (PE runs mixed bf16 x fp8 at the same column rate)
    xn = logits / np.maximum(
        np.sqrt(np.sum(logits * logits, axis=1, keepdims=True)), EPS)
    wn2 = np.sqrt(np.einsum("gce,gce->gc", weight, weight))[:, :, None]
    wn = weight / np.maximum(wn2, EPS)
    wq = (WS * wn).astype(FP8)                    # (G, C, E) fp8 table
    xb = xn.astype(BF16)                          # (B, E) bf16

    # per-sample margin + swap folded into (a, b):
    #   lb = ln(a * sum_c exp(ESCALE*psum_c) + b)
    #      = -64*ft - SHIFT + ln(sumexp with target swapped to 64*ft)
    # s_mm mimics the device's value of the target column: bf16-x times
    # fp8-w dot in f32, then the bf16 rounding of the PSUM->SBUF copy.
    wq_tar = wq[group, local].astype(np.float32)            # (B, E)
    psum_h = np.einsum("be,be->b", xb.astype(np.float32), wq_tar)
    s_mm = psum_h.astype(BF16).astype(np.float64) * ESCALE  # = 64*t_mm
    t = np.einsum("be,be->b", xn, wn[group, local]).astype(np.float64)
    sin_t = np.sqrt(np.clip(1.0 - t * t, 0.0, None))
    ft = np.where(t > THETA, t * COS_M - sin_t * SIN_M, t - SINMM)
    ft = np.where(labels != -1, ft, t)
    a64 = np.exp(-SCALE * ft - LB_SHIFT)
    b64 = (np.exp(SCALE * ft) - np.exp(s_mm)) * a64
    a32 = a64.astype(np.float32)
    b32 = b64.astype(np.float32)

    # band assignment: per (core, local-group), ceil(count/NG) bands
    percg = [[np.nonzero((core == c) & (gl == g))[0] for g in range(GPC)]
             for c in range(NCORES)]
    nbands = [sum(max(1, -(-len(idx) // NG)) for idx in percg[c])
              for c in range(NCORES)]
    nb = max(nbands)
    nb = -(-nb // BPT) * BPT  # round up to full sample tiles
    T = nb // BPT
    NP2 = nb // 2
    NPC = NP2 * NCC

    in_maps = []
    valid_rows = []
    for c in range(NCORES):
        # band -> (group, sample indices)
        bands = []
        for g in range(GPC):
            idx = percg[c][g]
            nslice = max(1, -(-len(idx) // NG))
            for s in range(nslice):
                bands.append((g, idx[s * NG:(s + 1) * NG]))
        while len(bands) < nb:
            bands.append((0, np.empty(0, dtype=np.int64)))

        xbp = np.zeros((T, 128, E), dtype=BF16)
        ab = np.ones((128, 2 * T), dtype=np.float32)
        valid = np.zeros((128, T), dtype=bool)
        # wt[p, cc*NP2 + b//2, b%2, k*512+c] = wq[band b][512cc+c, 128k+p]
        wt = np.empty((128, NPC, 2, KE * 512), dtype=FP8)
        for b, (g, idx) in enumerate(bands):
            wg = wq[c * GPC + g]                     # (C, E) fp8
            # warr[p, k, c] = wg[c, 128k+p]
            warr = np.ascontiguousarray(wg.reshape(C, KE, 128).transpose(2, 1, 0))
            for cc in range(NCC):
                wt[:, cc * NP2 + b // 2, b % 2] = warr[
                    :, :, 512 * cc:512 * cc + 512].reshape(128, KE * 512)
            ti, j = b // BPT, b % BPT
            sl = slice(NG * j, NG * j + len(idx))
            xbp[ti, sl, :] = xb[idx]
            ab[sl, ti] = a32[idx]
            ab[sl, T + ti] = b32[idx]
            valid[sl, ti] = True
        # xtq[p, 256t+128k+r] = xbp[t][r, 128k+p] (transposed PE stationary x)
        xtq = np.ascontiguousarray(np.transpose(
            xbp.reshape(T, 128, KE, 128), (3, 0, 2, 1))).reshape(128, T * KE * 128)
        in_maps.append({"wt": wt, "xtq": xtq, "ab": ab})
        valid_rows.append(valid)
    return in_maps, nb, valid_rows


def _run(logits, labels, weight, trace=False, **kw):
    from concourse.bass_utils import run_bass_kernel_spmd

    in_maps, nb, valid_rows = _pack(logits, labels, weight)
    nc = _graph_cache.get(nb)
    if nc is None:
        nc = _build(nb)
        _graph_cache[nb] = nc
    res = run_bass_kernel_spmd(nc, in_maps, core_ids=list(range(NCORES)),
                               trace=trace, **kw)
    total = sum(
        float(np.asarray(res.results[i]["out"], dtype=np.float32)[valid_rows[i]].sum())
        for i in range(NCORES)) / B + LB_SHIFT
    return np.asarray(total, dtype=np.float32), res


def kernel(logits, labels, weight):
    loss, _ = _run(logits, labels, weight)
    return loss


# revision 22
# speedup vs baseline: 1.0308x; 1.0308x over previous
"""ArcFace multi-head-sharded loss on 8 TRN2 NeuronCores.

Strategy: shard the (64, 2048, 256) weight table over the group axis —
each core owns 8 groups. Samples are routed host-side to the core owning
their group (host routing replaces the all-to-all). Weight rows are
l2-normalized host-side and quantized to fp8e4 (x16 pre-scale to stay in
the normal range); embeddings stay bf16 (PE runs mixed bf16 x fp8 at
full column rate). The device does:

  - stream its 8 weight groups (4MB fp8) from HBM on the sync HWDGE ring
    as 16 uniform 256KB pieces in chunk-major order (band-pair x
    512-class chunk, 2KB contiguous per partition), so each (tile,
    chunk) unlocks progressively and the compute pipeline chases the
    stream with a ~3us tail,
  - mains: psum(b, c) = <x_b, wq_c> on PE. Four 32-sample bands sit in
    the four column quadrants of the array (tile_position); the j-loop
    is innermost within each (chunk, k) group so the four quadrant
    matmuls overlap (~4ns issue stagger). A short warm-up burst flips
    the HAM clock gate to 2.4 GHz before the mains arrive,
  - exp per 512-class chunk with the class-axis sum fused via accum_out
    (scale folds the 1/16 weight quantization scale and the ArcFace
    scale 64),
  - lb = ln(a*sum + b) where (a, b) fold the entire ArcFace margin +
    target-logit swap, precomputed host-side per sample,
  - one [128,1] f32 column out per sample tile (all but the last tile's
    output DMA hide under the weight stream).

Host: sums the lb of valid rows across cores, /B + SHIFT. ~4MB HBM
traffic per core => memory-bound.

Samples are packed into "bands" of NG=32 partition rows, one band per
weight group (plus overflow bands), 4 bands per 128-row sample tile.
"""

import sys
import numpy as np
import ml_dtypes

BF16 = ml_dtypes.bfloat16
FP8 = ml_dtypes.float8_e4m3

_TRN_REPO = "/opt/trn_rl_repo"
if _TRN_REPO not in sys.path:
    sys.path.insert(0, _TRN_REPO)

# problem config (hardcoded per spec)
B, E, G, C = 512, 256, 64, 2048
NCORES = 8
GPC = G // NCORES        # weight groups per core
NG = 32                  # sample slots per band
BPT = 128 // NG          # bands per 128-partition sample tile
KE = E // 128            # contraction chunks
NCC = C // 512           # 512-col chunks per group
SCALE = 64.0
MARGIN = 0.5
COS_M = float(np.cos(MARGIN))
SIN_M = float(np.sin(MARGIN))
THETA = float(np.cos(np.pi - MARGIN))
SINMM = float(np.sin(np.pi - MARGIN) * MARGIN)
EPS = 1e-12
WS = 16.0                # fp8 weight pre-scale; PSUM = WS * cos (x is bf16)
ESCALE = SCALE / WS
NWARM = 8                # PE warm-up matmuls (HAM throttle release)

_graph_cache = {}


def _build(nb):
    """Build the per-core Bass graph for nb weight bands (nb % BPT == 0)."""
    from contextlib import ExitStack
    import concourse.bacc as bacc
    import concourse.tile as tile
    from concourse import mybir

    f32 = mybir.dt.float32
    bf16 = mybir.dt.bfloat16
    fp8 = mybir.dt.float8e4
    AF = mybir.ActivationFunctionType

    T = nb // BPT
    NP2 = nb // 2            # band pairs
    NPC = NP2 * NCC          # 256KB stream pieces, chunk-major
    nc = bacc.Bacc(None)

    # weight stream: piece i = (chunk cc = i // NP2, band pair pr = i % NP2),
    # per partition one flat 2KB run (so the DMA emits 2KB descriptors)
    wt_ext = nc.declare_dram_parameter("wt", [128, NPC, 2 * KE * 512], fp8,
                                       isOutput=False)
    xtq_ext = nc.declare_dram_parameter("xtq", [128, T * KE * 128], bf16,
                                        isOutput=False)
    ab_ext = nc.declare_dram_parameter("ab", [128, T], f32, isOutput=False)
    out_ext = nc.declare_dram_parameter("out", [128, T], f32, isOutput=True)

    with tile.TileContext(nc) as tc, ExitStack() as ctx:
        wpool = ctx.enter_context(tc.tile_pool(name="w", bufs=NPC))
        cpool = ctx.enter_context(tc.tile_pool(name="const", bufs=1))
        vpool = ctx.enter_context(tc.tile_pool(name="vec", bufs=2))
        pmain = ctx.enter_context(tc.tile_pool(name="pmain", bufs=8, space="PSUM"))

        p_tiles = [wpool.tile([128, 2 * KE * 512], fp8, tag="wt", name=f"wt{i}")
                   for i in range(NPC)]
        xtq_sb = cpool.tile([128, T * KE * 128], bf16, tag="xtq")
        ab_sb = cpool.tile([128, T], f32, tag="ab")
        lb_sb = cpool.tile([128, T], f32, tag="lb")

        # tiny inputs ride the scalar HWDGE ring so the weight stream on
        # the sync ring is never interrupted
        nc.scalar.dma_start(out=xtq_sb[:], in_=xtq_ext[:])
        nc.scalar.dma_start(out=ab_sb[:], in_=ab_ext[:])

        # the weight stream, chunk-major so (tile, chunk) compute unlocks
        # progressively while later chunks are still in flight
        for i in range(NPC):
            nc.sync.dma_start(out=p_tiles[i][:], in_=wt_ext[:, i])

        # exp scale as a per-partition AP (matches the fast ACT path; an
        # immediate scale measured ~2x slower per column)
        escale = cpool.tile([128, 1], f32, tag="escale")
        nc.vector.memset(escale[:], ESCALE)

        cps = {(t, cc): pmain.tile([128, 512], f32, tag="cos",
                                   name=f"cos{t}_{cc}")
               for t in range(T) for cc in range(NCC)}
        ses_t = {t: cpool.tile([128, NCC], f32, tag=f"ses{t}", name=f"ses{t}")
                 for t in range(T)}

        # PE warm-up: stream bf16 columns through the array so the HAM
        # activity window flips the clock gate to 2.4 GHz before the real
        # mains arrive (~3.4 us of sustained activity needed)
        if NWARM:
            warm_s = cpool.tile([128, 1], bf16, tag="warm_s")
            warm_m = cpool.tile([128, 512], bf16, tag="warm_m")
            nc.vector.memset(warm_s[:], 0.0)
            nc.vector.memset(warm_m[:], 0.0)
            for _ in range(NWARM):
                nc.tensor.matmul(cps[(0, 0)][0:1, :], warm_s[:], warm_m[:],
                                 start=True, stop=True)

        def mm(t, cc, k, j):
            pr = 2 * t + j // 2          # band pair of band 4t+j
            piece = p_tiles[cc * NP2 + pr]
            off = (j % 2) * KE * 512 + k * 512
            nc.tensor.matmul(
                cps[(t, cc)][NG * j:NG * (j + 1), :],
                xtq_sb[:, 256 * t + 128 * k + NG * j:
                       256 * t + 128 * k + NG * (j + 1)],
                piece[:, off:off + 512],
                start=(k == 0), stop=(k == KE - 1),
                tile_position=(0, NG * j),
            )

        def emit_exp(t, cc):
            """PSUM chunk -> bf16 SBUF via DVE (frees the bank, feeds ACT
            its fast input path), then exp with the class-axis sum fused
            via accum_out."""
            cb = cpool.tile([128, 512], bf16, tag=f"cosbf{t}_{cc}",
                            name=f"cosbf{t}_{cc}")
            nc.vector.tensor_copy(cb[:], cps[(t, cc)][:])
            escr = vpool.tile([128, 512], bf16, tag="escr")
            nc.scalar.activation(escr[:], cb[:], AF.Exp, scale=escale[:],
                                 accum_out=ses_t[t][:, cc:cc + 1])

        def emit_tail(t):
            """v = sum_cc ses + b; b folds the target-logit swap. The host
            takes ln(v) - 64*ft (no device Ln -> only exp uses the ACT
            tables, so the compiler never toggles table sets mid-kernel).
            Both ops ride DVE back-to-back (no cross-engine hop)."""
            sfull = cpool.tile([128, 1], f32, tag=f"sfull{t}")
            nc.vector.reduce_sum(sfull[:], ses_t[t][:], axis=mybir.AxisListType.X)
            nc.vector.tensor_tensor(lb_sb[:, t:t + 1], sfull[:],
                                    ab_sb[:, t:t + 1], mybir.AluOpType.add)
            nc.scalar.dma_start(out=out_ext[:, t:t + 1], in_=lb_sb[:, t:t + 1])

        # chunk-major emission matching the stream order: (t, cc) chunks
        # complete every other piece; j innermost within each (cc, k) so
        # the four quadrant matmuls overlap (~4ns issue stagger)
        for cc in range(NCC):
            for t in range(T):
                for k in range(KE):
                    for j in range(BPT):
                        mm(t, cc, k, j)
                emit_exp(t, cc)
                if cc == NCC - 1:
                    emit_tail(t)

    nc.compile()
    return nc


def _pack(logits, labels, weight):
    """Route samples to the core owning their group; build per-core inputs."""
    logits = np.asarray(logits, dtype=np.float32)
    labels = np.asarray(labels).astype(np.int64)
    weight = np.asarray(weight, dtype=np.float32)

    group = (labels // C).astype(np.int64)
    local = (labels % C).astype(np.int64)
    core = group // GPC
    gl = group % GPC

    # host-side l2 normalization; weights quantized to fp8 (x16 keeps
    # values in fp8e4's normal range; cos is invariant to row scaling),
    # x stays bf16 (PE runs mixed bf16 x fp8 at the same column rate)
    xn = logits / np.maximum(
        np.sqrt(np.sum(logits * logits, axis=1, keepdims=True)), EPS)
    wn2 = np.sqrt(np.einsum("gce,gce->gc", weight, weight))[:, :, None]
    wn = weight / np.maximum(wn2, EPS)
    wq = (WS * wn).astype(FP8)                    # (G, C, E) fp8 table
    xb = xn.astype(BF16)                          # (B, E) bf16

    # per-sample target-logit swap folded into b:
    #   v = sum_c exp(ESCALE*psum_c) + b,  b = exp(64*ft) - exp(64*t_mm)
    # => ln(v) = logsumexp with the target column swapped to 64*ft.
    # The host then computes loss_i = ln(v) - 64*ft (margin fully host-side).
    # s_mm mimics the device's value of the target column: bf16-x times
    # fp8-w dot in f32, then the bf16 rounding of the PSUM->SBUF copy.
    wq_tar = wq[group, local].astype(np.float32)            # (B, E)
    psum_h = np.einsum("be,be->b", xb.astype(np.float32), wq_tar)
    s_mm = psum_h.astype(BF16).astype(np.float64) * ESCALE  # = 64*t_mm
    t = np.einsum("be,be->b", xn, wn[group, local]).astype(np.float64)
    sin_t = np.sqrt(np.clip(1.0 - t * t, 0.0, None))
    ft = np.where(t > THETA, t * COS_M - sin_t * SIN_M, t - SINMM)
    ft = np.where(labels != -1, ft, t)
    sft = SCALE * ft
    b32 = (np.exp(sft) - np.exp(s_mm)).astype(np.float32)

    # band assignment: per (core, local-group), ceil(count/NG) bands
    percg = [[np.nonzero((core == c) & (gl == g))[0] for g in range(GPC)]
             for c in range(NCORES)]
    nbands = [sum(max(1, -(-len(idx) // NG)) for idx in percg[c])
              for c in range(NCORES)]
    nb = max(nbands)
    nb = -(-nb // BPT) * BPT  # round up to full sample tiles
    T = nb // BPT
    NP2 = nb // 2
    NPC = NP2 * NCC

    in_maps = []
    valid_rows = []
    for c in range(NCORES):
        # band -> (group, sample indices)
        bands = []
        for g in range(GPC):
            idx = percg[c][g]
            nslice = max(1, -(-len(idx) // NG))
            for s in range(nslice):
                bands.append((g, idx[s * NG:(s + 1) * NG]))
        while len(bands) < nb:
            bands.append((0, np.empty(0, dtype=np.int64)))

        xbp = np.zeros((T, 128, E), dtype=BF16)
        ab = np.zeros((128, T), dtype=np.float32)
        sftm = np.zeros((128, T), dtype=np.float64)
        valid = np.zeros((128, T), dtype=bool)
        # wt[p, cc*NP2 + b//2, (b%2)*KE*512 + k*512 + c]
        #   = wq[band b][512cc+c, 128k+p]
        wt = np.empty((128, NPC, 2 * KE * 512), dtype=FP8)
        wtv = wt.reshape(128, NPC, 2, KE * 512)
        for b, (g, idx) in enumerate(bands):
            wg = wq[c * GPC + g]                     # (C, E) fp8
            # warr[p, k, c] = wg[c, 128k+p]
            warr = np.ascontiguousarray(wg.reshape(C, KE, 128).transpose(2, 1, 0))
            for cc in range(NCC):
                wtv[:, cc * NP2 + b // 2, b % 2] = warr[
                    :, :, 512 * cc:512 * cc + 512].reshape(128, KE * 512)
            ti, j = b // BPT, b % BPT
            sl = slice(NG * j, NG * j + len(idx))
            xbp[ti, sl, :] = xb[idx]
            ab[sl, ti] = b32[idx]
            sftm[sl, ti] = sft[idx]
            valid[sl, ti] = True
        # xtq[p, 256t+128k+r] = xbp[t][r, 128k+p] (transposed PE stationary x)
        xtq = np.ascontiguousarray(np.transpose(
            xbp.reshape(T, 128, KE, 128), (3, 0, 2, 1))).reshape(128, T * KE * 128)
        in_maps.append({"wt": wt, "xtq": xtq, "ab": ab})
        valid_rows.append((valid, sftm))
    return in_maps, nb, valid_rows


def _run(logits, labels, weight, trace=False, **kw):
    from concourse.bass_utils import run_bass_kernel_spmd

    in_maps, nb, valid_rows = _pack(logits, labels, weight)
    nc = _graph_cache.get(nb)
    if nc is None:
        nc = _build(nb)
        _graph_cache[nb] = nc
    res = run_bass_kernel_spmd(nc, in_maps, core_ids=list(range(NCORES)),
                               trace=trace, **kw)
    total = 0.0
    for i in range(NCORES):
        valid, sftm = valid_rows[i]
        v = np.asarray(res.results[i]["out"], dtype=np.float32).astype(np.float64)
        total += float((np.log(v[valid]) - sftm[valid]).sum())
    total /= B
    return np.asarray(total, dtype=np.float32), res


def kernel(logits, labels, weight):
    loss, _ = _run(logits, labels, weight)
    return loss


# revision 30
# speedup vs baseline: 1.0588x; 1.0271x over previous
"""ArcFace multi-head-sharded loss on 8 TRN2 NeuronCores.

Strategy: shard the (64, 2048, 256) weight table over the group axis —
each core owns 8 groups. Samples are routed host-side to the core owning
their group (host routing replaces the all-to-all). Weight rows are
l2-normalized host-side and quantized to fp8e4 (x16 pre-scale to stay in
the normal range); embeddings stay bf16 (PE runs mixed bf16 x fp8 at
full column rate). The device does:

  - stream its 8 weight groups (4MB fp8) from HBM on the sync HWDGE ring
    as 16 uniform 256KB pieces in chunk-major order (band-pair x
    512-class chunk, 2KB contiguous per partition), so each (tile,
    chunk) unlocks progressively and the compute pipeline chases the
    stream with a ~3us tail,
  - mains: psum(b, c) = <x_b, wq_c> on PE. Four 32-sample bands sit in
    the four column quadrants of the array (tile_position); the j-loop
    is innermost within each (chunk, k) group so the four quadrant
    matmuls overlap (~4ns issue stagger). A short warm-up burst flips
    the HAM clock gate to 2.4 GHz before the mains arrive,
  - exp per 512-class chunk with the class-axis sum fused via accum_out
    (scale folds the 1/16 weight quantization scale and the ArcFace
    scale 64),
  - lb = ln(a*sum + b) where (a, b) fold the entire ArcFace margin +
    target-logit swap, precomputed host-side per sample,
  - one [128,1] f32 column out per sample tile (all but the last tile's
    output DMA hide under the weight stream).

Host: sums the lb of valid rows across cores, /B + SHIFT. ~4MB HBM
traffic per core => memory-bound.

Samples are packed into "bands" of NG=32 partition rows, one band per
weight group (plus overflow bands), 4 bands per 128-row sample tile.
"""

import sys
import numpy as np
import ml_dtypes

BF16 = ml_dtypes.bfloat16
FP8 = ml_dtypes.float8_e4m3

_TRN_REPO = "/opt/trn_rl_repo"
if _TRN_REPO not in sys.path:
    sys.path.insert(0, _TRN_REPO)

# problem config (hardcoded per spec)
B, E, G, C = 512, 256, 64, 2048
NCORES = 8
GPC = G // NCORES        # weight groups per core
NG = 32                  # sample slots per band
BPT = 128 // NG          # bands per 128-partition sample tile
KE = E // 128            # contraction chunks
NCC = C // 512           # 512-col chunks per group
SCALE = 64.0
MARGIN = 0.5
COS_M = float(np.cos(MARGIN))
SIN_M = float(np.sin(MARGIN))
THETA = float(np.cos(np.pi - MARGIN))
SINMM = float(np.sin(np.pi - MARGIN) * MARGIN)
EPS = 1e-12
WS = 16.0                # fp8 weight pre-scale; PSUM = WS * cos (x is bf16)
ESCALE = SCALE / WS
NWARM = 8                # PE warm-up matmuls (HAM throttle release)

_graph_cache = {}


def _build(nb):
    """Build the per-core Bass graph for nb weight bands (nb % BPT == 0)."""
    from contextlib import ExitStack
    import concourse.bacc as bacc
    import concourse.tile as tile
    from concourse import mybir

    f32 = mybir.dt.float32
    bf16 = mybir.dt.bfloat16
    fp8 = mybir.dt.float8e4
    AF = mybir.ActivationFunctionType

    T = nb // BPT
    NPC = T * NCC            # 512KB stream pieces: one full (tile, chunk)
    PSZ = BPT * KE * 512     # piece bytes per partition (4KB descriptors)
    nc = bacc.Bacc(None)

    # weight stream: piece i = cc * T + t holds all BPT bands of tile t
    # for 512-class chunk cc, one flat 4KB run per partition (big DMA
    # descriptors keep the drain rate near the HBM roofline)
    wt_ext = nc.declare_dram_parameter("wt", [128, NPC, PSZ], fp8,
                                       isOutput=False)
    xtq_ext = nc.declare_dram_parameter("xtq", [128, T * KE * 128], bf16,
                                        isOutput=False)
    ab_ext = nc.declare_dram_parameter("ab", [128, T], f32, isOutput=False)
    out_ext = nc.declare_dram_parameter("out", [128, T], f32, isOutput=True)

    with tile.TileContext(nc) as tc, ExitStack() as ctx:
        wpool = ctx.enter_context(tc.tile_pool(name="w", bufs=NPC))
        cpool = ctx.enter_context(tc.tile_pool(name="const", bufs=1))
        vpool = ctx.enter_context(tc.tile_pool(name="vec", bufs=2))
        pmain = ctx.enter_context(tc.tile_pool(name="pmain", bufs=8, space="PSUM"))

        p_tiles = [wpool.tile([128, PSZ], fp8, tag="wt", name=f"wt{i}")
                   for i in range(NPC)]
        xtq_sb = cpool.tile([128, T * KE * 128], bf16, tag="xtq")
        ab_sb = cpool.tile([128, T], f32, tag="ab")
        lb_sb = cpool.tile([128, T], f32, tag="lb")

        # tiny inputs ride the scalar HWDGE ring so the weight stream on
        # the sync ring is never interrupted
        nc.scalar.dma_start(out=xtq_sb[:], in_=xtq_ext[:])
        nc.scalar.dma_start(out=ab_sb[:], in_=ab_ext[:])

        # the weight stream, chunk-major so (tile, chunk) compute unlocks
        # progressively while later chunks are still in flight
        for i in range(NPC):
            nc.sync.dma_start(out=p_tiles[i][:], in_=wt_ext[:, i])

        # exp scale as a per-partition AP (matches the fast ACT path; an
        # immediate scale measured ~2x slower per column)
        escale = cpool.tile([128, 1], f32, tag="escale")
        nc.vector.memset(escale[:], ESCALE)

        cps = {(t, cc): pmain.tile([128, 512], f32, tag="cos",
                                   name=f"cos{t}_{cc}")
               for t in range(T) for cc in range(NCC)}
        ses_t = {t: cpool.tile([128, NCC], f32, tag=f"ses{t}", name=f"ses{t}")
                 for t in range(T)}

        # PE warm-up: stream bf16 columns through the array so the HAM
        # activity window flips the clock gate to 2.4 GHz before the real
        # mains arrive (~3.4 us of sustained activity needed)
        if NWARM:
            warm_s = cpool.tile([128, 1], bf16, tag="warm_s")
            warm_m = cpool.tile([128, 512], bf16, tag="warm_m")
            nc.vector.memset(warm_s[:], 0.0)
            nc.vector.memset(warm_m[:], 0.0)
            for _ in range(NWARM):
                nc.tensor.matmul(cps[(0, 0)][0:1, :], warm_s[:], warm_m[:],
                                 start=True, stop=True)

        def mm(t, cc, k, j):
            piece = p_tiles[cc * T + t]
            off = j * KE * 512 + k * 512
            nc.tensor.matmul(
                cps[(t, cc)][NG * j:NG * (j + 1), :],
                xtq_sb[:, 256 * t + 128 * k + NG * j:
                       256 * t + 128 * k + NG * (j + 1)],
                piece[:, off:off + 512],
                start=(k == 0), stop=(k == KE - 1),
                tile_position=(0, NG * j),
            )

        def emit_exp(t, cc):
            """exp over the chunk with the class-axis sum fused via
            accum_out. Normally PSUM -> bf16 SBUF via DVE first (frees the
            bank, feeds ACT its fast input path); the very last chunk reads
            PSUM directly so its end-of-kernel chain skips the cast."""
            escr = vpool.tile([128, 512], bf16, tag="escr")
            if (t, cc) == (T - 1, NCC - 1):
                nc.scalar.activation(escr[:], cps[(t, cc)][:], AF.Exp,
                                     scale=escale[:],
                                     accum_out=ses_t[t][:, cc:cc + 1])
            else:
                cb = cpool.tile([128, 512], bf16, tag=f"cosbf{t}_{cc}",
                                name=f"cosbf{t}_{cc}")
                nc.vector.tensor_copy(cb[:], cps[(t, cc)][:])
                nc.scalar.activation(escr[:], cb[:], AF.Exp, scale=escale[:],
                                     accum_out=ses_t[t][:, cc:cc + 1])

        def emit_tail(t):
            """v = sum_cc ses + b; b folds the target-logit swap. The host
            takes ln(v) - 64*ft (no device Ln -> only exp uses the ACT
            tables, so the compiler never toggles table sets mid-kernel).
            Both ops ride DVE back-to-back (no cross-engine hop)."""
            sfull = cpool.tile([128, 1], f32, tag=f"sfull{t}")
            nc.vector.reduce_sum(sfull[:], ses_t[t][:], axis=mybir.AxisListType.X)
            nc.vector.tensor_tensor(lb_sb[:, t:t + 1], sfull[:],
                                    ab_sb[:, t:t + 1], mybir.AluOpType.add)
            # output DMA rides the (long idle) sync ring so its trigger
            # never queues ahead of the last chunk's exp on the ACT queue
            nc.sync.dma_start(out=out_ext[:, t:t + 1], in_=lb_sb[:, t:t + 1])

        # chunk-major emission matching the stream order: (t, cc) chunks
        # complete every other piece; j innermost within each (cc, k) so
        # the four quadrant matmuls overlap (~4ns issue stagger)
        for cc in range(NCC):
            for t in range(T):
                for k in range(KE):
                    for j in range(BPT):
                        mm(t, cc, k, j)
                emit_exp(t, cc)
                if cc == NCC - 1:
                    emit_tail(t)

    nc.compile()
    return nc


def _pack(logits, labels, weight):
    """Route samples to the core owning their group; build per-core inputs."""
    logits = np.asarray(logits, dtype=np.float32)
    labels = np.asarray(labels).astype(np.int64)
    weight = np.asarray(weight, dtype=np.float32)

    group = (labels // C).astype(np.int64)
    local = (labels % C).astype(np.int64)
    core = group // GPC
    gl = group % GPC

    # host-side l2 normalization; weights quantized to fp8 (x16 keeps
    # values in fp8e4's normal range; cos is invariant to row scaling),
    # x stays bf16 (PE runs mixed bf16 x fp8 at the same column rate)
    xn = logits / np.maximum(
        np.sqrt(np.sum(logits * logits, axis=1, keepdims=True)), EPS)
    wn2 = np.sqrt(np.einsum("gce,gce->gc", weight, weight))[:, :, None]
    wn = weight / np.maximum(wn2, EPS)
    wq = (WS * wn).astype(FP8)                    # (G, C, E) fp8 table
    xb = xn.astype(BF16)                          # (B, E) bf16

    # per-sample target-logit swap folded into b:
    #   v = sum_c exp(ESCALE*psum_c) + b,  b = exp(64*ft) - exp(64*t_mm)
    # => ln(v) = logsumexp with the target column swapped to 64*ft.
    # The host then computes loss_i = ln(v) - 64*ft (margin fully host-side).
    # s_mm mimics the device's value of the target column: bf16-x times
    # fp8-w dot in f32, then the bf16 rounding of the PSUM->SBUF copy.
    wq_tar = wq[group, local].astype(np.float32)            # (B, E)
    psum_h = np.einsum("be,be->b", xb.astype(np.float32), wq_tar)
    s_mm = psum_h.astype(BF16).astype(np.float64) * ESCALE  # = 64*t_mm
    t = np.einsum("be,be->b", xn, wn[group, local]).astype(np.float64)
    sin_t = np.sqrt(np.clip(1.0 - t * t, 0.0, None))
    ft = np.where(t > THETA, t * COS_M - sin_t * SIN_M, t - SINMM)
    ft = np.where(labels != -1, ft, t)
    sft = SCALE * ft
    b32 = (np.exp(sft) - np.exp(s_mm)).astype(np.float32)

    # band assignment: per (core, local-group), ceil(count/NG) bands
    percg = [[np.nonzero((core == c) & (gl == g))[0] for g in range(GPC)]
             for c in range(NCORES)]
    nbands = [sum(max(1, -(-len(idx) // NG)) for idx in percg[c])
              for c in range(NCORES)]
    nb = max(nbands)
    nb = -(-nb // BPT) * BPT  # round up to full sample tiles
    T = nb // BPT
    NPC = T * NCC
    PSZ = BPT * KE * 512

    in_maps = []
    valid_rows = []
    for c in range(NCORES):
        # band -> (group, sample indices)
        bands = []
        for g in range(GPC):
            idx = percg[c][g]
            nslice = max(1, -(-len(idx) // NG))
            for s in range(nslice):
                bands.append((g, idx[s * NG:(s + 1) * NG]))
        while len(bands) < nb:
            bands.append((0, np.empty(0, dtype=np.int64)))

        xbp = np.zeros((T, 128, E), dtype=BF16)
        ab = np.zeros((128, T), dtype=np.float32)
        sftm = np.zeros((128, T), dtype=np.float64)
        valid = np.zeros((128, T), dtype=bool)
        # wt[p, cc*T + t, j*KE*512 + k*512 + c]
        #   = wq[band 4t+j][512cc+c, 128k+p]
        wt = np.empty((128, NPC, PSZ), dtype=FP8)
        wtv = wt.reshape(128, NPC, BPT, KE * 512)
        for b, (g, idx) in enumerate(bands):
            wg = wq[c * GPC + g]                     # (C, E) fp8
            # warr[p, k, c] = wg[c, 128k+p]
            warr = np.ascontiguousarray(wg.reshape(C, KE, 128).transpose(2, 1, 0))
            for cc in range(NCC):
                wtv[:, cc * T + b // BPT, b % BPT] = warr[
                    :, :, 512 * cc:512 * cc + 512].reshape(128, KE * 512)
            ti, j = b // BPT, b % BPT
            sl = slice(NG * j, NG * j + len(idx))
            xbp[ti, sl, :] = xb[idx]
            ab[sl, ti] = b32[idx]
            sftm[sl, ti] = sft[idx]
            valid[sl, ti] = True
        # xtq[p, 256t+128k+r] = xbp[t][r, 128k+p] (transposed PE stationary x)
        xtq = np.ascontiguousarray(np.transpose(
            xbp.reshape(T, 128, KE, 128), (3, 0, 2, 1))).reshape(128, T * KE * 128)
        in_maps.append({"wt": wt, "xtq": xtq, "ab": ab})
        valid_rows.append((valid, sftm))
    return in_maps, nb, valid_rows


def _run(logits, labels, weight, trace=False, **kw):
    from concourse.bass_utils import run_bass_kernel_spmd

    in_maps, nb, valid_rows = _pack(logits, labels, weight)
    nc = _graph_cache.get(nb)
    if nc is None:
        nc = _build(nb)
        _graph_cache[nb] = nc
    res = run_bass_kernel_spmd(nc, in_maps, core_ids=list(range(NCORES)),
                               trace=trace, **kw)
    total = 0.0
    for i in range(NCORES):
        valid, sftm = valid_rows[i]
        v = np.asarray(res.results[i]["out"], dtype=np.float32).astype(np.float64)
        total += float((np.log(v[valid]) - sftm[valid]).sum())
    total /= B
    return np.asarray(total, dtype=np.float32), res


def kernel(logits, labels, weight):
    loss, _ = _run(logits, labels, weight)
    return loss
